# revision 13
# baseline (speedup 1.0000x reference)
"""Trainium2 Bass kernel for the ClassCaps vote-transform problem.

Math (from the reference):
    votes[b,h,w,i,o,m,p] = sum_n poses[b,h,w,i,m,n] * weight[i,o,n,p]
    votes[...,0,3] += xv[h,w];  votes[...,1,3] += yv[h,w]
    outputs: votes reshaped to (b,1,1,1,1,h*w*i,o,4,4), acts = activations
             reshaped to (b,1,1,1,1,h*w*i,1,1,1)

Strategy: pure data parallel over the batch (8 of 64 per core). Per core the
work is a per-site [16 -> 160] linear map. We express it as K=128 block-
diagonal matmuls: stationary = transposed poses for 128 sites x 8 capsules
(rows = (capsule, m*4+n)), moving = an expanded block-diagonal weight
[128, 8*160], so one matmul group emits votes for 8 capsules of 128 sites in
the exact DRAM layout. The host pre-packs the transposed/expanded arrays
(layout only, no math); coordinates are added with two strided per-partition
tensor_scalar ops on-chip.
"""

import os
import sys

import numpy as np

_TRN_REPO = "/opt/trn_rl_repo"
if _TRN_REPO not in sys.path:
    sys.path.insert(0, _TRN_REPO)

# Problem constants (hardcoded per the task contract).
B, H, W, CAPS_IN, A, CAPS_OUT = 64, 14, 14, 32, 4, 10
N_CORES = 8
B_CORE = B // N_CORES                  # 8 batch elements per core
HW = H * W                             # 196
SITES = B_CORE * HW                    # 1568 (b,hw) sites per core
BLK = 128                              # sites per block
N_BLK = (SITES + BLK - 1) // BLK       # 13 (12 full + 1 of 32)
SITES_PAD = N_BLK * BLK                # 1664
GRP = 4                                # capsule groups of 8 per block
CAPS_G = CAPS_IN // GRP                # 8 capsules per group
MN = A * A                             # 16
OMP = CAPS_OUT * MN                    # 160 output values per (site, capsule)
GCOLS = CAPS_G * OMP                   # 1280 votes columns per group
VCOLS = CAPS_IN * OMP                  # 5120 votes columns per site
ACOLS = SITES * CAPS_IN // BLK         # 392 acts columns per partition

# const tensor column layout: [wexp | xyrs | acts]
C_W0 = 0
C_XY0 = GRP * GCOLS                    # 5120
C_AC0 = C_XY0 + 2 * N_BLK              # 5146
C_COLS = C_AC0 + ACOLS                 # 5538

# Matmul input dtype: "float32" (exact, 4 cyc/col) or "float32r" (1 cyc/col).
COMPUTE_DT = os.environ.get("CLASSCAPS_MM_DTYPE", "float32")

LAST_RESULTS = None  # BassKernelResults of the most recent run (for test.py)


def _split_bf16(x):
    """Error-free-ish split x ~= hi + lo with hi, lo bf16."""
    import ml_dtypes

    hi = x.astype(ml_dtypes.bfloat16)
    lo = (x - hi.astype(np.float32)).astype(ml_dtypes.bfloat16)
    return hi, lo


def _pack_inputs_eft(poses, activations, xv, yv, weight):
    """bf16 EFT packing: per capsule K=48 rows [Phi; Plo; Phi] against
    rhs [Whi; Whi; Wlo]; 2 capsules per 128-row stationary group."""
    import ml_dtypes

    bf16 = ml_dtypes.bfloat16
    poses = np.ascontiguousarray(poses, dtype=np.float32)
    activations = np.ascontiguousarray(activations, dtype=np.float32)
    xv = np.ascontiguousarray(xv, dtype=np.float32).reshape(HW)
    yv = np.ascontiguousarray(yv, dtype=np.float32).reshape(HW)
    weight = np.ascontiguousarray(weight, dtype=np.float32)

    p = poses.reshape(N_CORES, SITES, CAPS_IN, MN)
    p_pad = np.zeros((N_CORES, SITES_PAD, CAPS_IN, MN), dtype=np.float32)
    p_pad[:, :SITES] = p
    ph, pl = _split_bf16(p_pad)
    # [core, blk, site, g2(16), il(2), mn] -> rows (term,16)x3 stacked per il
    G2 = CAPS_IN // 2  # 16 groups of 2 capsules
    vh = ph.reshape(N_CORES, N_BLK, BLK, G2, 2, MN)
    vl = pl.reshape(N_CORES, N_BLK, BLK, G2, 2, MN)
    pT = np.zeros((N_CORES, BLK, N_BLK * G2 * BLK), dtype=bf16)
    pt4 = pT.reshape(N_CORES, BLK, N_BLK, G2, BLK)
    for il in range(2):
        r0 = 48 * il
        # transpose [core, blk, site, g2, mn] -> [core, mn, blk, g2, site]
        pt4[:, r0 : r0 + 16] = vh[:, :, :, :, il].transpose(0, 4, 1, 3, 2)
        pt4[:, r0 + 16 : r0 + 32] = vl[:, :, :, :, il].transpose(0, 4, 1, 3, 2)
        pt4[:, r0 + 32 : r0 + 48] = pt4[:, r0 : r0 + 16]

    wh, wl = _split_bf16(weight)  # [i, o, n, p]
    wexp = np.zeros((G2, 128, 2, CAPS_OUT, A, A), dtype=bf16)  # [g2, row, il2, o, m2, p]
    whg = wh.reshape(G2, 2, CAPS_OUT, A, A)
    wlg = wl.reshape(G2, 2, CAPS_OUT, A, A)
    for il in range(2):
        r0 = 48 * il
        for m in range(A):
            # rows (il, term, m, n) -> cols (il, o, m, p)
            wexp[:, r0 + 4 * m : r0 + 4 * m + 4, il, :, m, :] = whg[:, il].transpose(
                0, 2, 1, 3
            )
            wexp[:, r0 + 16 + 4 * m : r0 + 20 + 4 * m, il, :, m, :] = wlg[
                :, il
            ].transpose(0, 2, 1, 3)
            # third term: Phi rows against Wlo
            wexp[:, r0 + 32 + 4 * m : r0 + 36 + 4 * m, il, :, m, :] = wlg[
                :, il
            ].transpose(0, 2, 1, 3)
    # fix: second term rows (Plo) pair with Whi, third (Phi dup) with Wlo
    for il in range(2):
        r0 = 48 * il
        for m in range(A):
            wexp[:, r0 + 16 + 4 * m : r0 + 20 + 4 * m, il, :, m, :] = whg[
                :, il
            ].transpose(0, 2, 1, 3)
    wexp = wexp.reshape(G2, BLK, 2 * OMP).transpose(1, 0, 2).reshape(BLK, G2 * 2 * OMP)

    xs = np.zeros(SITES_PAD, dtype=np.float32)
    ys = np.zeros(SITES_PAD, dtype=np.float32)
    xs[:SITES] = np.tile(xv, B_CORE)
    ys[:SITES] = np.tile(yv, B_CORE)
    xyrs = np.empty((BLK, 2 * N_BLK), dtype=np.float32)
    xyrs[:, 0::2] = xs.reshape(N_BLK, BLK).T
    xyrs[:, 1::2] = ys.reshape(N_BLK, BLK).T

    acts = activations.reshape(N_CORES, BLK, ACOLS)

    in_maps = []
    for c in range(N_CORES):
        cst = np.empty((BLK, 2 * N_BLK + ACOLS), dtype=np.float32)
        cst[:, : 2 * N_BLK] = xyrs
        cst[:, 2 * N_BLK :] = acts[c]
        in_maps.append(
            {
                "pt": np.ascontiguousarray(pT[c]),
                "wexp": np.ascontiguousarray(wexp),
                "cst": cst,
            }
        )
    return in_maps


def _pack_inputs(poses, activations, xv, yv, weight):
    """Host-side sharding/layout prep (pure layout, no math)."""
    poses = np.ascontiguousarray(poses, dtype=np.float32)
    activations = np.ascontiguousarray(activations, dtype=np.float32)
    xv = np.ascontiguousarray(xv, dtype=np.float32).reshape(HW)
    yv = np.ascontiguousarray(yv, dtype=np.float32).reshape(HW)
    weight = np.ascontiguousarray(weight, dtype=np.float32)

    # [core, site, i, mn] with the site dim zero-padded to 13*128
    p = poses.reshape(N_CORES, SITES, CAPS_IN, MN)
    p_pad = np.zeros((N_CORES, SITES_PAD, CAPS_IN, MN), dtype=np.float32)
    p_pad[:, :SITES] = p
    # pT[core, row=(il,mn), col=(blk, grp, site)]
    v = p_pad.reshape(N_CORES, N_BLK, BLK, GRP, CAPS_G, MN)
    pT = (
        v.transpose(0, 4, 5, 1, 3, 2)  # [core, il, mn, blk, grp, site]
        .reshape(N_CORES, BLK, N_BLK * GRP * BLK)
        .copy()
    )

    # Block-diagonal expanded weight wexp[row=(il, m, n), col=(g, il2, o, m2, p)]
    wg = weight.reshape(GRP, CAPS_G, CAPS_OUT, A, A)  # [g, il, o, n, p]
    wexp = np.zeros((GRP, CAPS_G, A, A, CAPS_G, CAPS_OUT, A, A), dtype=np.float32)
    for il in range(CAPS_G):
        for m in range(A):
            # rows (il, m, n) -> cols (il, o, m, p): value weight[8g+il, o, n, p]
            wexp[:, il, m, :, il, :, m, :] = wg[:, il].transpose(0, 2, 1, 3)
    wexp = wexp.reshape(GRP, BLK, GCOLS).transpose(1, 0, 2).reshape(BLK, GRP * GCOLS)

    # Per-site coordinate scalars, laid out [partition, 2*blk]
    xs = np.zeros(SITES_PAD, dtype=np.float32)
    ys = np.zeros(SITES_PAD, dtype=np.float32)
    xs[:SITES] = np.tile(xv, B_CORE)
    ys[:SITES] = np.tile(yv, B_CORE)
    xyrs = np.empty((BLK, 2 * N_BLK), dtype=np.float32)
    xyrs[:, 0::2] = xs.reshape(N_BLK, BLK).T
    xyrs[:, 1::2] = ys.reshape(N_BLK, BLK).T

    acts = activations.reshape(N_CORES, BLK, ACOLS)

    in_maps = []
    for c in range(N_CORES):
        cst = np.empty((BLK, C_COLS), dtype=np.float32)
        cst[:, C_W0:C_XY0] = wexp
        cst[:, C_XY0:C_AC0] = xyrs
        cst[:, C_AC0:] = acts[c]
        in_maps.append({"pt": pT[c], "cst": cst})
    return in_maps


def _build_nc():
    from contextlib import ExitStack

    import concourse.bass as bass
    import concourse.tile as tile
    from concourse import bacc, mybir

    f32 = mybir.dt.float32
    mm_dt = getattr(mybir.dt, COMPUTE_DT)

    # Bacc (not raw Bass): its compile() legalizes multi-wait instructions
    # (move_matmul_waits_to_ldweights + generate_event_semaphores) into
    # ISA-legal single-wait sequences.
    nc = bacc.Bacc("TRN2", target_bir_lowering=False, debug=False)

    if COMPUTE_DT == "bf16_eft":
        return _build_nc_eft(nc, bass, tile, mybir)

    pt_d = nc.dram_tensor("pt", [BLK, N_BLK * GRP * BLK], f32, kind="ExternalInput").ap()
    cst_d = nc.dram_tensor("cst", [BLK, C_COLS], f32, kind="ExternalInput").ap()
    votes_d = nc.dram_tensor("votes", [SITES, VCOLS], f32, kind="ExternalOutput").ap()
    actso_d = nc.dram_tensor("acts_out", [BLK, ACOLS], f32, kind="ExternalOutput").ap()

    with ExitStack() as ctx:
        tc = ctx.enter_context(tile.TileContext(nc))
        const_pool = ctx.enter_context(tc.tile_pool(name="const", bufs=1))
        in_pool = ctx.enter_context(tc.tile_pool(name="inp", bufs=3))
        stage_pool = ctx.enter_context(tc.tile_pool(name="stage", bufs=2))
        psum_pool = ctx.enter_context(tc.tile_pool(name="ps", bufs=2, space="PSUM"))
        dummy_pool = ctx.enter_context(tc.tile_pool(name="dmy", bufs=1, space="PSUM"))

        cst_sb = const_pool.tile([BLK, C_COLS], f32, tag="cst")
        nc.sync.dma_start(cst_sb[:], cst_d[:])
        wexp_sb = cst_sb[:, C_W0:C_XY0]
        xy_sb = cst_sb[:, C_XY0:C_AC0]

        for j in range(N_BLK):
            m = min(BLK, SITES - j * BLK)  # sites in this block
            pt_sb = in_pool.tile([BLK, GRP * BLK], f32, tag="pt")
            nc.sync.dma_start(pt_sb[:], pt_d[:, GRP * BLK * j : GRP * BLK * (j + 1)])

            stage = stage_pool.tile([BLK, VCOLS], f32, tag="stage")
            for g in range(GRP):
                # 1536 cols = 3 full PSUM banks, so slots never share a bank
                ps = psum_pool.tile([BLK, 1536], f32, tag="ps")
                lhsT = pt_sb[:, BLK * g : BLK * g + m].bitcast(mm_dt)
                for c0, n in ((0, 512), (512, 512), (1024, 256)):
                    nc.tensor.matmul(
                        ps[:m, c0 : c0 + n],
                        lhsT,
                        wexp_sb[:, GCOLS * g + c0 : GCOLS * g + c0 + n].bitcast(mm_dt),
                        start=True,
                        stop=True,
                    )
                if g % 2 == 0:
                    nc.vector.tensor_copy(
                        stage[:m, GCOLS * g : GCOLS * (g + 1)], ps[:m, :GCOLS]
                    )
                else:
                    nc.scalar.copy(
                        stage[:m, GCOLS * g : GCOLS * (g + 1)], ps[:m, :GCOLS]
                    )

            # votes[..., 0, 3] += xv ; votes[..., 1, 3] += yv (per-partition scalars)
            vview = stage[:m].rearrange(
                "p (i o m q) -> p i o m q", i=CAPS_IN, o=CAPS_OUT, m=A, q=A
            )
            nc.vector.tensor_scalar_add(
                vview[:, :, :, 0, 3], vview[:, :, :, 0, 3], xy_sb[:m, 2 * j : 2 * j + 1]
            )
            nc.vector.tensor_scalar_add(
                vview[:, :, :, 1, 3], vview[:, :, :, 1, 3], xy_sb[:m, 2 * j + 1 : 2 * j + 2]
            )

            nc.sync.dma_start(votes_d[BLK * j : BLK * j + m, :], stage[:m, :])

        # acts passthrough (pure reshape in the reference)
        nc.sync.dma_start(actso_d[:], cst_sb[:, C_AC0:])

    nc.compile()
    return nc


def _build_nc_eft(nc, bass, tile, mybir):
    """bf16 EFT variant: 16 matmuls/block of K=96, N=320 (2 capsules each)."""
    from contextlib import ExitStack

    f32 = mybir.dt.float32
    bf16 = mybir.dt.bfloat16
    G2 = CAPS_IN // 2

    pt_d = nc.dram_tensor("pt", [BLK, N_BLK * G2 * BLK], bf16, kind="ExternalInput").ap()
    wexp_d = nc.dram_tensor("wexp", [BLK, G2 * 2 * OMP], bf16, kind="ExternalInput").ap()
    cst_d = nc.dram_tensor("cst", [BLK, 2 * N_BLK + ACOLS], f32, kind="ExternalInput").ap()
    votes_d = nc.dram_tensor("votes", [SITES, VCOLS], f32, kind="ExternalOutput").ap()
    actso_d = nc.dram_tensor("acts_out", [BLK, ACOLS], f32, kind="ExternalOutput").ap()

    with ExitStack() as ctx:
        tc = ctx.enter_context(tile.TileContext(nc))
        const_pool = ctx.enter_context(tc.tile_pool(name="const", bufs=1))
        in_pool = ctx.enter_context(tc.tile_pool(name="inp", bufs=3))
        stage_pool = ctx.enter_context(tc.tile_pool(name="stage", bufs=2))
        psum_pool = ctx.enter_context(tc.tile_pool(name="ps", bufs=8, space="PSUM"))

        wexp_sb = const_pool.tile([BLK, G2 * 2 * OMP], bf16, tag="wexp")
        nc.sync.dma_start(wexp_sb[:], wexp_d[:])
        cst_sb = const_pool.tile([BLK, 2 * N_BLK + ACOLS], f32, tag="cst")
        nc.sync.dma_start(cst_sb[:], cst_d[:])
        xy_sb = cst_sb[:, : 2 * N_BLK]

        for j in range(N_BLK):
            m = min(BLK, SITES - j * BLK)
            pt_sb = in_pool.tile([BLK, G2 * BLK], bf16, tag="pt")
            nc.sync.dma_start(pt_sb[:], pt_d[:, G2 * BLK * j : G2 * BLK * (j + 1)])

            stage = stage_pool.tile([BLK, VCOLS], f32, tag="stage")
            for g2 in range(G2):
                ps = psum_pool.tile([BLK, 512], f32, tag="ps")
                nc.tensor.matmul(
                    ps[:m, : 2 * OMP],
                    pt_sb[:96, BLK * g2 : BLK * g2 + m],
                    wexp_sb[:96, 2 * OMP * g2 : 2 * OMP * (g2 + 1)],
                    start=True,
                    stop=True,
                )
                dst = stage[:m, 2 * OMP * g2 : 2 * OMP * (g2 + 1)]
                if g2 % 2 == 0:
                    nc.vector.tensor_copy(dst, ps[:m, : 2 * OMP])
                else:
                    nc.scalar.copy(dst, ps[:m, : 2 * OMP])

            vview = stage[:m].rearrange(
                "p (i o m q) -> p i o m q", i=CAPS_IN, o=CAPS_OUT, m=A, q=A
            )
            nc.vector.tensor_scalar_add(
                vview[:, :, :, 0, 3], vview[:, :, :, 0, 3], xy_sb[:m, 2 * j : 2 * j + 1]
            )
            nc.vector.tensor_scalar_add(
                vview[:, :, :, 1, 3], vview[:, :, :, 1, 3], xy_sb[:m, 2 * j + 1 : 2 * j + 2]
            )
            nc.sync.dma_start(votes_d[BLK * j : BLK * j + m, :], stage[:m, :])

        nc.sync.dma_start(actso_d[:], cst_sb[:, 2 * N_BLK :])

    nc.compile()
    return nc


_NC_CACHE = None


def kernel(poses, activations, weight, xv, yv):
    global LAST_RESULTS, _NC_CACHE
    from concourse.bass_utils import run_bass_kernel_spmd

    if COMPUTE_DT == "bf16_eft":
        in_maps = _pack_inputs_eft(poses, activations, xv, yv, weight)
    else:
        in_maps = _pack_inputs(poses, activations, xv, yv, weight)
    if _NC_CACHE is None:
        _NC_CACHE = _build_nc()
    nc = _NC_CACHE

    res = run_bass_kernel_spmd(nc, in_maps, core_ids=list(range(N_CORES)))
    LAST_RESULTS = res

    votes = np.concatenate(
        [r["votes"].reshape(B_CORE, 1, 1, 1, 1, HW * CAPS_IN, CAPS_OUT, A, A) for r in res.results],
        axis=0,
    )
    acts = np.concatenate(
        [r["acts_out"].reshape(B_CORE, 1, 1, 1, 1, HW * CAPS_IN, 1, 1, 1) for r in res.results],
        axis=0,
    )
    return votes, acts


# revision 14
# speedup vs baseline: 1.0079x; 1.0079x over previous
"""Trainium2 Bass kernel for the ClassCaps vote-transform problem.

Math (from the reference):
    votes[b,h,w,i,o,m,p] = sum_n poses[b,h,w,i,m,n] * weight[i,o,n,p]
    votes[...,0,3] += xv[h,w];  votes[...,1,3] += yv[h,w]
    outputs: votes reshaped to (b,1,1,1,1,h*w*i,o,4,4), acts = activations
             reshaped to (b,1,1,1,1,h*w*i,1,1,1)

Strategy: pure data parallel over the batch (8 of 64 per core). Per core the
work is a per-site [16 -> 160] linear map. We express it as K=128 block-
diagonal matmuls: stationary = transposed poses for 128 sites x 8 capsules
(rows = (capsule, m*4+n)), moving = an expanded block-diagonal weight
[128, 8*160], so one matmul group emits votes for 8 capsules of 128 sites in
the exact DRAM layout. The host pre-packs the transposed/expanded arrays
(layout only, no math); coordinates are added with two strided per-partition
tensor_scalar ops on-chip.
"""

import os
import sys

import numpy as np

_TRN_REPO = "/opt/trn_rl_repo"
if _TRN_REPO not in sys.path:
    sys.path.insert(0, _TRN_REPO)

# Problem constants (hardcoded per the task contract).
B, H, W, CAPS_IN, A, CAPS_OUT = 64, 14, 14, 32, 4, 10
N_CORES = 8
B_CORE = B // N_CORES                  # 8 batch elements per core
HW = H * W                             # 196
SITES = B_CORE * HW                    # 1568 (b,hw) sites per core
BLK = 128                              # sites per block
N_BLK = (SITES + BLK - 1) // BLK       # 13 (12 full + 1 of 32)
SITES_PAD = N_BLK * BLK                # 1664
GRP = 4                                # capsule groups of 8 per block
CAPS_G = CAPS_IN // GRP                # 8 capsules per group
MN = A * A                             # 16
OMP = CAPS_OUT * MN                    # 160 output values per (site, capsule)
GCOLS = CAPS_G * OMP                   # 1280 votes columns per group
VCOLS = CAPS_IN * OMP                  # 5120 votes columns per site
ACOLS = SITES * CAPS_IN // BLK         # 392 acts columns per partition

# const tensor column layout: [wexp | xyrs | acts]
C_W0 = 0
C_XY0 = GRP * GCOLS                    # 5120
C_AC0 = C_XY0 + 2 * N_BLK              # 5146
C_COLS = C_AC0 + ACOLS                 # 5538

# Matmul input dtype: "float32" (exact, 4 cyc/col) or "float32r" (1 cyc/col).
COMPUTE_DT = os.environ.get("CLASSCAPS_MM_DTYPE", "float32")

LAST_RESULTS = None  # BassKernelResults of the most recent run (for test.py)


def _split_bf16(x):
    """Error-free-ish split x ~= hi + lo with hi, lo bf16."""
    import ml_dtypes

    hi = x.astype(ml_dtypes.bfloat16)
    lo = (x - hi.astype(np.float32)).astype(ml_dtypes.bfloat16)
    return hi, lo


def _pack_inputs_eft(poses, activations, xv, yv, weight):
    """bf16 EFT packing: per capsule K=48 rows [Phi; Plo; Phi] against
    rhs [Whi; Whi; Wlo]; 2 capsules per 128-row stationary group."""
    import ml_dtypes

    bf16 = ml_dtypes.bfloat16
    poses = np.ascontiguousarray(poses, dtype=np.float32)
    activations = np.ascontiguousarray(activations, dtype=np.float32)
    xv = np.ascontiguousarray(xv, dtype=np.float32).reshape(HW)
    yv = np.ascontiguousarray(yv, dtype=np.float32).reshape(HW)
    weight = np.ascontiguousarray(weight, dtype=np.float32)

    p = poses.reshape(N_CORES, SITES, CAPS_IN, MN)
    p_pad = np.zeros((N_CORES, SITES_PAD, CAPS_IN, MN), dtype=np.float32)
    p_pad[:, :SITES] = p
    ph, pl = _split_bf16(p_pad)
    # [core, blk, site, g2(16), il(2), mn] -> rows (term,16)x3 stacked per il
    G2 = CAPS_IN // 2  # 16 groups of 2 capsules
    vh = ph.reshape(N_CORES, N_BLK, BLK, G2, 2, MN)
    vl = pl.reshape(N_CORES, N_BLK, BLK, G2, 2, MN)
    pT = np.zeros((N_CORES, BLK, N_BLK * G2 * BLK), dtype=bf16)
    pt4 = pT.reshape(N_CORES, BLK, N_BLK, G2, BLK)
    for il in range(2):
        r0 = 48 * il
        # transpose [core, blk, site, g2, mn] -> [core, mn, blk, g2, site]
        pt4[:, r0 : r0 + 16] = vh[:, :, :, :, il].transpose(0, 4, 1, 3, 2)
        pt4[:, r0 + 16 : r0 + 32] = vl[:, :, :, :, il].transpose(0, 4, 1, 3, 2)
        pt4[:, r0 + 32 : r0 + 48] = pt4[:, r0 : r0 + 16]

    wh, wl = _split_bf16(weight)  # [i, o, n, p]
    wexp = np.zeros((G2, 128, 2, CAPS_OUT, A, A), dtype=bf16)  # [g2, row, il2, o, m2, p]
    whg = wh.reshape(G2, 2, CAPS_OUT, A, A)
    wlg = wl.reshape(G2, 2, CAPS_OUT, A, A)
    for il in range(2):
        r0 = 48 * il
        for m in range(A):
            # rows (il, term, m, n) -> cols (il, o, m, p)
            wexp[:, r0 + 4 * m : r0 + 4 * m + 4, il, :, m, :] = whg[:, il].transpose(
                0, 2, 1, 3
            )
            wexp[:, r0 + 16 + 4 * m : r0 + 20 + 4 * m, il, :, m, :] = wlg[
                :, il
            ].transpose(0, 2, 1, 3)
            # third term: Phi rows against Wlo
            wexp[:, r0 + 32 + 4 * m : r0 + 36 + 4 * m, il, :, m, :] = wlg[
                :, il
            ].transpose(0, 2, 1, 3)
    # fix: second term rows (Plo) pair with Whi, third (Phi dup) with Wlo
    for il in range(2):
        r0 = 48 * il
        for m in range(A):
            wexp[:, r0 + 16 + 4 * m : r0 + 20 + 4 * m, il, :, m, :] = whg[
                :, il
            ].transpose(0, 2, 1, 3)
    wexp = wexp.reshape(G2, BLK, 2 * OMP).transpose(1, 0, 2).reshape(BLK, G2 * 2 * OMP)

    xs = np.zeros(SITES_PAD, dtype=np.float32)
    ys = np.zeros(SITES_PAD, dtype=np.float32)
    xs[:SITES] = np.tile(xv, B_CORE)
    ys[:SITES] = np.tile(yv, B_CORE)
    xyrs = np.empty((BLK, 2 * N_BLK), dtype=np.float32)
    xyrs[:, 0::2] = xs.reshape(N_BLK, BLK).T
    xyrs[:, 1::2] = ys.reshape(N_BLK, BLK).T

    acts = activations.reshape(N_CORES, BLK, ACOLS)

    in_maps = []
    for c in range(N_CORES):
        cst = np.empty((BLK, 2 * N_BLK + ACOLS), dtype=np.float32)
        cst[:, : 2 * N_BLK] = xyrs
        cst[:, 2 * N_BLK :] = acts[c]
        in_maps.append(
            {
                "pt": np.ascontiguousarray(pT[c]),
                "wexp": np.ascontiguousarray(wexp),
                "cst": cst,
            }
        )
    return in_maps


def _pack_inputs(poses, activations, xv, yv, weight):
    """Host-side sharding/layout prep (pure layout, no math)."""
    poses = np.ascontiguousarray(poses, dtype=np.float32)
    activations = np.ascontiguousarray(activations, dtype=np.float32)
    xv = np.ascontiguousarray(xv, dtype=np.float32).reshape(HW)
    yv = np.ascontiguousarray(yv, dtype=np.float32).reshape(HW)
    weight = np.ascontiguousarray(weight, dtype=np.float32)

    # [core, site, i, mn] with the site dim zero-padded to 13*128
    p = poses.reshape(N_CORES, SITES, CAPS_IN, MN)
    p_pad = np.zeros((N_CORES, SITES_PAD, CAPS_IN, MN), dtype=np.float32)
    p_pad[:, :SITES] = p
    # pT[core, row=(il,mn), col=(blk, grp, site)]
    v = p_pad.reshape(N_CORES, N_BLK, BLK, GRP, CAPS_G, MN)
    pT = (
        v.transpose(0, 4, 5, 1, 3, 2)  # [core, il, mn, blk, grp, site]
        .reshape(N_CORES, BLK, N_BLK * GRP * BLK)
        .copy()
    )

    # Block-diagonal expanded weight wexp[row=(il, m, n), col=(g, il2, o, m2, p)]
    wg = weight.reshape(GRP, CAPS_G, CAPS_OUT, A, A)  # [g, il, o, n, p]
    wexp = np.zeros((GRP, CAPS_G, A, A, CAPS_G, CAPS_OUT, A, A), dtype=np.float32)
    for il in range(CAPS_G):
        for m in range(A):
            # rows (il, m, n) -> cols (il, o, m, p): value weight[8g+il, o, n, p]
            wexp[:, il, m, :, il, :, m, :] = wg[:, il].transpose(0, 2, 1, 3)
    wexp = wexp.reshape(GRP, BLK, GCOLS).transpose(1, 0, 2).reshape(BLK, GRP * GCOLS)

    # Per-site coordinate scalars, laid out [partition, 2*blk]
    xs = np.zeros(SITES_PAD, dtype=np.float32)
    ys = np.zeros(SITES_PAD, dtype=np.float32)
    xs[:SITES] = np.tile(xv, B_CORE)
    ys[:SITES] = np.tile(yv, B_CORE)
    xyrs = np.empty((BLK, 2 * N_BLK), dtype=np.float32)
    xyrs[:, 0::2] = xs.reshape(N_BLK, BLK).T
    xyrs[:, 1::2] = ys.reshape(N_BLK, BLK).T

    acts = activations.reshape(N_CORES, BLK, ACOLS)

    in_maps = []
    for c in range(N_CORES):
        cst = np.empty((BLK, C_COLS), dtype=np.float32)
        cst[:, C_W0:C_XY0] = wexp
        cst[:, C_XY0:C_AC0] = xyrs
        cst[:, C_AC0:] = acts[c]
        in_maps.append({"pt": pT[c], "cst": cst})
    return in_maps


def _build_nc():
    from contextlib import ExitStack

    import concourse.bass as bass
    import concourse.tile as tile
    from concourse import bacc, mybir

    # Bacc (not raw Bass): its compile() legalizes multi-wait instructions
    # (move_matmul_waits_to_ldweights + generate_event_semaphores) into
    # ISA-legal single-wait sequences.
    nc = bacc.Bacc("TRN2", target_bir_lowering=False, debug=False)

    if COMPUTE_DT == "bf16_eft":
        return _build_nc_eft(nc, bass, tile, mybir)

    f32 = mybir.dt.float32
    mm_dt = getattr(mybir.dt, COMPUTE_DT)

    pt_d = nc.dram_tensor("pt", [BLK, N_BLK * GRP * BLK], f32, kind="ExternalInput").ap()
    cst_d = nc.dram_tensor("cst", [BLK, C_COLS], f32, kind="ExternalInput").ap()
    votes_d = nc.dram_tensor("votes", [SITES, VCOLS], f32, kind="ExternalOutput").ap()
    actso_d = nc.dram_tensor("acts_out", [BLK, ACOLS], f32, kind="ExternalOutput").ap()

    with ExitStack() as ctx:
        tc = ctx.enter_context(tile.TileContext(nc))
        const_pool = ctx.enter_context(tc.tile_pool(name="const", bufs=1))
        in_pool = ctx.enter_context(tc.tile_pool(name="inp", bufs=3))
        stage_pool = ctx.enter_context(tc.tile_pool(name="stage", bufs=2))
        psum_pool = ctx.enter_context(tc.tile_pool(name="ps", bufs=2, space="PSUM"))
        dummy_pool = ctx.enter_context(tc.tile_pool(name="dmy", bufs=1, space="PSUM"))

        cst_sb = const_pool.tile([BLK, C_COLS], f32, tag="cst")
        nc.sync.dma_start(cst_sb[:], cst_d[:])
        wexp_sb = cst_sb[:, C_W0:C_XY0]
        xy_sb = cst_sb[:, C_XY0:C_AC0]

        for j in range(N_BLK):
            m = min(BLK, SITES - j * BLK)  # sites in this block
            pt_sb = in_pool.tile([BLK, GRP * BLK], f32, tag="pt")
            nc.sync.dma_start(pt_sb[:], pt_d[:, GRP * BLK * j : GRP * BLK * (j + 1)])

            stage = stage_pool.tile([BLK, VCOLS], f32, tag="stage")
            for g in range(GRP):
                # 1536 cols = 3 full PSUM banks, so slots never share a bank
                ps = psum_pool.tile([BLK, 1536], f32, tag="ps")
                lhsT = pt_sb[:, BLK * g : BLK * g + m].bitcast(mm_dt)
                for c0, n in ((0, 512), (512, 512), (1024, 256)):
                    nc.tensor.matmul(
                        ps[:m, c0 : c0 + n],
                        lhsT,
                        wexp_sb[:, GCOLS * g + c0 : GCOLS * g + c0 + n].bitcast(mm_dt),
                        start=True,
                        stop=True,
                    )
                if g % 2 == 0:
                    nc.vector.tensor_copy(
                        stage[:m, GCOLS * g : GCOLS * (g + 1)], ps[:m, :GCOLS]
                    )
                else:
                    nc.scalar.copy(
                        stage[:m, GCOLS * g : GCOLS * (g + 1)], ps[:m, :GCOLS]
                    )

            # votes[..., 0, 3] += xv ; votes[..., 1, 3] += yv (per-partition scalars)
            vview = stage[:m].rearrange(
                "p (i o m q) -> p i o m q", i=CAPS_IN, o=CAPS_OUT, m=A, q=A
            )
            nc.vector.tensor_scalar_add(
                vview[:, :, :, 0, 3], vview[:, :, :, 0, 3], xy_sb[:m, 2 * j : 2 * j + 1]
            )
            nc.vector.tensor_scalar_add(
                vview[:, :, :, 1, 3], vview[:, :, :, 1, 3], xy_sb[:m, 2 * j + 1 : 2 * j + 2]
            )

            nc.sync.dma_start(votes_d[BLK * j : BLK * j + m, :], stage[:m, :])

        # acts passthrough (pure reshape in the reference)
        nc.sync.dma_start(actso_d[:], cst_sb[:, C_AC0:])

    nc.compile()
    return nc


def _build_nc_eft(nc, bass, tile, mybir):
    """bf16 EFT variant: 16 matmuls/block of K=96, N=320 (2 capsules each)."""
    from contextlib import ExitStack

    f32 = mybir.dt.float32
    bf16 = mybir.dt.bfloat16
    G2 = CAPS_IN // 2

    pt_d = nc.dram_tensor("pt", [BLK, N_BLK * G2 * BLK], bf16, kind="ExternalInput").ap()
    wexp_d = nc.dram_tensor("wexp", [BLK, G2 * 2 * OMP], bf16, kind="ExternalInput").ap()
    cst_d = nc.dram_tensor("cst", [BLK, 2 * N_BLK + ACOLS], f32, kind="ExternalInput").ap()
    votes_d = nc.dram_tensor("votes", [SITES, VCOLS], f32, kind="ExternalOutput").ap()
    actso_d = nc.dram_tensor("acts_out", [BLK, ACOLS], f32, kind="ExternalOutput").ap()

    with ExitStack() as ctx:
        tc = ctx.enter_context(tile.TileContext(nc))
        const_pool = ctx.enter_context(tc.tile_pool(name="const", bufs=1))
        in_pool = ctx.enter_context(tc.tile_pool(name="inp", bufs=3))
        stage_pool = ctx.enter_context(tc.tile_pool(name="stage", bufs=2))
        psum_pool = ctx.enter_context(tc.tile_pool(name="ps", bufs=8, space="PSUM"))

        wexp_sb = const_pool.tile([BLK, G2 * 2 * OMP], bf16, tag="wexp")
        nc.sync.dma_start(wexp_sb[:], wexp_d[:])
        cst_sb = const_pool.tile([BLK, 2 * N_BLK + ACOLS], f32, tag="cst")
        nc.sync.dma_start(cst_sb[:], cst_d[:])
        xy_sb = cst_sb[:, : 2 * N_BLK]

        for j in range(N_BLK):
            m = min(BLK, SITES - j * BLK)
            pt_sb = in_pool.tile([BLK, G2 * BLK], bf16, tag="pt")
            nc.sync.dma_start(pt_sb[:], pt_d[:, G2 * BLK * j : G2 * BLK * (j + 1)])

            stage = stage_pool.tile([BLK, VCOLS], f32, tag="stage")
            for g2 in range(G2):
                ps = psum_pool.tile([BLK, 512], f32, tag="ps")
                nc.tensor.matmul(
                    ps[:m, : 2 * OMP],
                    pt_sb[:96, BLK * g2 : BLK * g2 + m],
                    wexp_sb[:96, 2 * OMP * g2 : 2 * OMP * (g2 + 1)],
                    start=True,
                    stop=True,
                )
                dst = stage[:m, 2 * OMP * g2 : 2 * OMP * (g2 + 1)]
                if g2 % 2 == 0:
                    nc.vector.tensor_copy(dst, ps[:m, : 2 * OMP])
                else:
                    nc.scalar.copy(dst, ps[:m, : 2 * OMP])

            vview = stage[:m].rearrange(
                "p (i o m q) -> p i o m q", i=CAPS_IN, o=CAPS_OUT, m=A, q=A
            )
            nc.vector.tensor_scalar_add(
                vview[:, :, :, 0, 3], vview[:, :, :, 0, 3], xy_sb[:m, 2 * j : 2 * j + 1]
            )
            nc.vector.tensor_scalar_add(
                vview[:, :, :, 1, 3], vview[:, :, :, 1, 3], xy_sb[:m, 2 * j + 1 : 2 * j + 2]
            )
            nc.sync.dma_start(votes_d[BLK * j : BLK * j + m, :], stage[:m, :])

        nc.sync.dma_start(actso_d[:], cst_sb[:, 2 * N_BLK :])

    nc.compile()
    return nc


_NC_CACHE = None


def kernel(poses, activations, weight, xv, yv):
    global LAST_RESULTS, _NC_CACHE
    from concourse.bass_utils import run_bass_kernel_spmd

    if COMPUTE_DT == "bf16_eft":
        in_maps = _pack_inputs_eft(poses, activations, xv, yv, weight)
    else:
        in_maps = _pack_inputs(poses, activations, xv, yv, weight)
    if _NC_CACHE is None:
        _NC_CACHE = _build_nc()
    nc = _NC_CACHE

    res = run_bass_kernel_spmd(nc, in_maps, core_ids=list(range(N_CORES)))
    LAST_RESULTS = res

    votes = np.concatenate(
        [r["votes"].reshape(B_CORE, 1, 1, 1, 1, HW * CAPS_IN, CAPS_OUT, A, A) for r in res.results],
        axis=0,
    )
    acts = np.concatenate(
        [r["acts_out"].reshape(B_CORE, 1, 1, 1, 1, HW * CAPS_IN, 1, 1, 1) for r in res.results],
        axis=0,
    )
    return votes, acts
